# revision 43
# baseline (speedup 1.0000x reference)
"""Trainium2 Bass kernel for nn_AdaptiveAttention (B=4, S=2048, D=1024, H=16, E=64).

Sharding: data-parallel over (batch, seq-half) -> 8 cores, zero collectives.
Each core handles 1024 query rows of one batch: projects q/k/v for all 16
heads (k/v projections duplicated between the two cores sharing a batch),
runs softmax attention with transposed-score layout (scoresT = [t, s]), and
applies the output Dense. head_weights = softmax(head_selector) folds into Wv
on the host; 1/sqrt(E) folds into Wq; bv/bo fold into one output bias row.

Attention runs one head at a time (scores -> exp on ScalarE -> probs@V with a
ones column appended to V so the 65th output row accumulates the softmax
denominator). The exp stream keeps ScalarE ~fully busy; the spare TensorE
cycles and the 2 spare PSUM banks are used to run the k/q/v projections for
later heads interleaved inside the attention loop.
"""

import numpy as np
import ml_dtypes

P = 128
D = 1024          # d_model
HE = 1024         # H*E
SQ = 1024         # query rows per core
SK = 2048         # key rows
H = 16
E = 64
NG = 8            # he-chunks of 128 (head pairs)
NDT = 8           # d-model 128-tiles
NT = 16           # key 128-tiles
B, S = 4, 2048

BF16 = ml_dtypes.bfloat16

_CACHE = {}


def _build():
    import concourse.mybir as mybir
    import concourse.tile as tile
    from concourse import bacc
    from concourse.bass import ds, ts

    DT = mybir.dt.bfloat16
    F32 = mybir.dt.float32
    Exp = mybir.ActivationFunctionType.Exp
    ADD = mybir.AluOpType.add
    MUL = mybir.AluOpType.mult

    nc = bacc.Bacc("TRN2", target_bir_lowering=False, debug=False,
                   enable_asserts=False, num_devices=8)

    qT_d = nc.dram_tensor("qT", [D, SQ], DT, kind="ExternalInput").ap()
    kT_d = nc.dram_tensor("kT", [D, SK], DT, kind="ExternalInput").ap()
    vT_d = nc.dram_tensor("vT", [D, SK], DT, kind="ExternalInput").ap()
    wq_d = nc.dram_tensor("wq", [NG, D, P], DT, kind="ExternalInput").ap()
    wk_d = nc.dram_tensor("wk", [NG, D, P], DT, kind="ExternalInput").ap()
    wv_d = nc.dram_tensor("wv", [D, HE], DT, kind="ExternalInput").ap()
    wo_d = nc.dram_tensor("wo", [HE, D], DT, kind="ExternalInput").ap()
    # packed: [bias_bc(1024) | bq(8) | bk(8) | zeros(16)]
    bias_d = nc.dram_tensor("biases", [P, D + 32], F32, kind="ExternalInput").ap()
    out_d = nc.dram_tensor("out", [SQ, D], F32, kind="ExternalOutput").ap()

    with tile.TileContext(nc) as tc:
        with tc.tile_pool(name="pers", bufs=1) as pers, \
             tc.tile_pool(name="instream", bufs=1) as instream, \
             tc.tile_pool(name="vinp", bufs=2) as vinp, \
             tc.tile_pool(name="wstream", bufs=2) as wstream, \
             tc.tile_pool(name="expp", bufs=3) as expp, \
             tc.tile_pool(name="bcp", bufs=2) as bcp, \
             tc.tile_pool(name="ctmpp", bufs=2) as ctmpp, \
             tc.tile_pool(name="osbp", bufs=2) as osbp, \
             tc.tile_pool(name="drp", bufs=2, space="DRAM") as drp, \
             tc.tile_pool(name="psS", bufs=2, space="PSUM") as psS, \
             tc.tile_pool(name="psC", bufs=1, space="PSUM") as psC, \
             tc.tile_pool(name="psP", bufs=1, space="PSUM") as psP:

            # ---- persistent tiles ----
            k_sb = pers.tile([P, NG, SK], DT)      # kT per he-chunk: [e, g, t]
            q_sb = pers.tile([P, NG, SQ], DT)      # qT per he-chunk: [e, g, s]
            # v with a ones column per head (65 cols/head)
            v_sb = pers.tile([P, NT, H * (E + 1)], DT)
            ctx_sb = pers.tile([P, NG, SQ], DT)    # normalized ctx^T: [e, g, s]
            ball = pers.tile([P, D + 32], F32)
            nc.sync.dma_start(ball[:], bias_d[:])
            bias_sb = ball[:, 0:D]
            bq_sb = ball[:, D : D + NG]
            bk_sb = ball[:, D + NG : D + 2 * NG]
            zb_sb = ball[:, D + 2 * NG : D + 2 * NG + 1]

            kT_r = kT_d.rearrange("(dt p) t -> p dt t", p=P)
            qT_r = qT_d.rearrange("(dt p) s -> p dt s", p=P)
            vT_r = vT_d.rearrange("(dt p) t -> p dt t", p=P)
            wv_r = wv_d.rearrange("(dt p) e -> p dt e", p=P)
            wo_r = wo_d.rearrange("(g p) d -> p g d", p=P)

            # ---- input staging: kin/qin stay live through the whole
            # attention (interleaved proj groups read them); wv -> wo chain.
            kin = instream.tile([P, NDT, SK], DT, tag="kin", name="kin")
            for dt_ in range(NDT):
                nc.sync.dma_start(kin[:, dt_], kT_r[:, dt_])

            v_head = v_sb.rearrange("p i (h c) -> p i h c", c=E + 1)
            nc.vector.memset(v_head[:, :, :, E : E + 1], 1.0)

            # ---------- projection work items (emitted now or interleaved) ----
            def emit_wdma(g, w_d):
                def _fn():
                    wt = wstream.tile([P, NDT, P], DT, tag="wk",
                                      name=f"w{g}_{w_d.tensor.name}")
                    nc.sync.dma_start(wt[:], w_d[g].rearrange("(dt p) e -> p dt e", p=P))
                    return wt
                return _fn

            def emit_kq_group(wt_ref, inp, out_sb, bias_ap, g, j2, nchunks, nm):
                # one psum pair: output column chunks (2*j2, 2*j2+1)
                def _fn():
                    wt = wt_ref[0]
                    pss = [psP.tile([P, 512], F32, tag=f"pp{j}",
                                    name=f"{nm}_{g}_{j2}_{j}")
                           for j in range(2)]
                    for dt_ in range(NDT):
                        for j in range(2):
                            cj = 2 * j2 + j
                            if cj >= nchunks:
                                continue
                            nc.tensor.matmul(pss[j][:], wt[:, dt_],
                                             inp[:, dt_, ds(cj * 512, 512)],
                                             start=dt_ == 0, stop=dt_ == NDT - 1)
                    for j in range(2):
                        cj = 2 * j2 + j
                        if cj >= nchunks:
                            continue
                        nc.vector.tensor_scalar(out_sb[:, g, ds(cj * 512, 512)],
                                                pss[j][:], bias_ap[:, g, None],
                                                None, op0=ADD)
                return _fn

            def emit_vin_dma(i, vin_ref):
                def _fn():
                    vin = vinp.tile([P, NDT, P], DT, tag="vin", name=f"vin_{i}")
                    nc.sync.dma_start(vin[:], vT_r[:, :, ds(i * P, P)])
                    vin_ref[0] = vin
                return _fn

            def emit_v_group(vin_ref, wv_ref, i, hh):
                def _fn():
                    vin, wvt = vin_ref[0], wv_ref[0]
                    ps = psP.tile([P, 512], F32, tag=f"pp{i % 2}", name=f"vp_{i}_{hh}")
                    for dt_ in range(NDT):
                        nc.tensor.matmul(ps[:], vin[:, dt_],
                                         wvt[:, dt_, ds(hh * 512, 512)],
                                         start=dt_ == 0, stop=dt_ == NDT - 1)
                    nc.vector.tensor_copy(
                        v_head[:, i, ds(hh * 8, 8), 0:E],
                        ps.rearrange("p (h c) -> p h c", c=E))
                return _fn

            # -------- prologue: k/q proj for g=0, v proj for heads 0-7 --------
            wk_ref, wq_ref, wv_ref = [None], [None], [None]
            vin_refs = {i: [None] for i in range(NT)}

            qin = instream.tile([P, NDT, SQ], DT, tag="qin", name="qin")
            for dh in range(2):
                nc.sync.dma_start(qin[:, ds(dh * 4, 4)], qT_r[:, ds(dh * 4, 4)])
            wv_sb = instream.tile([P, NDT, HE], DT, tag="wvo", name="wv_sb")
            for dh in range(2):
                nc.sync.dma_start(wv_sb[:, ds(dh * 4, 4)], wv_r[:, ds(dh * 4, 4)])
            wv_ref[0] = wv_sb

            wk_ref[0] = emit_wdma(0, wk_d)()
            emit_kq_group(wk_ref, kin, k_sb, bk_sb, 0, 0, 4, "kp")()
            wq_ref[0] = emit_wdma(0, wq_d)()
            emit_kq_group(wq_ref, qin, q_sb, bq_sb, 0, 0, 2, "qp")()
            for i in range(3):
                emit_vin_dma(i, vin_refs[i])()
                emit_v_group(vin_refs[i], wv_ref, i, 0)()

            wo_ref = [None]

            def emit_wo_dma():
                wo_t = instream.tile([P, NG, D], DT, tag="wvo", name="wo_sb")
                for gh in range(2):
                    nc.sync.dma_start(wo_t[:, ds(gh * 4, 4)], wo_r[:, ds(gh * 4, 4)])
                wo_ref[0] = wo_t

            # -------- interleaved work queue for heads 1..15 --------
            # due: kq(g) before head 2g; v(i, hh=1) before head 8.
            def mk_setref(ref, g, w_d):
                def _fn():
                    ref[0] = emit_wdma(g, w_d)()
                return _fn

            sched = {h: [] for h in range(H)}
            # head 0 (2 pops/iter): k(g0) chunks 2-3, then v(3..15, hh0) JIT
            sched[0].append(emit_kq_group(wk_ref, kin, k_sb, bk_sb, 0, 1, 4, "kp"))
            for i in range(3, NT):
                sched[0].append(emit_vin_dma(i, vin_refs[i]))
                sched[0].append(emit_v_group(vin_refs[i], wv_ref, i, 0))
            # v hh=1 spread over heads 1..6
            vitems = []
            for i in range(NT):
                vitems.append(emit_vin_dma(i, vin_refs[i]))
                vitems.append(emit_v_group(vin_refs[i], wv_ref, i, 1))
            for n, it in enumerate(vitems):
                sched[1 + min(n // 8, 3)].append(it)
            sched[7].append(emit_wo_dma)
            for g in range(1, NG):
                grp = [mk_setref(wk_ref, g, wk_d)] + \
                      [emit_kq_group(wk_ref, kin, k_sb, bk_sb, g, j2, 4, "kp")
                       for j2 in range(2)] + \
                      [mk_setref(wq_ref, g, wq_d),
                       emit_kq_group(wq_ref, qin, q_sb, bq_sb, g, 0, 2, "qp")]
                for n, it in enumerate(grp):
                    sched[2 * g - 2 + (n >= 3)].append(it)

            # -------- attention, one head at a time --------
            # process head 15 before head 14: the last head's normalize
            # chain is on the critical path into out-proj, and even-row heads
            # skip the btmp partition-shift DMA.
            HORDER = list(range(H - 2)) + [H - 1, H - 2]
            for hpos, h in enumerate(HORDER):
                g, rh = h // 2, (h % 2) * E
                queue = list(sched[hpos])
                cps = [psC.tile([E + 1, 512], F32, tag=f"c{sc}", name=f"c{sc}_{h}")
                       for sc in range(2)]
                vcols = ds(h * (E + 1), E + 1)
                e_prev = None
                for i in range(NT):
                    s_ = psS.tile([P, SQ], F32, tag="s", name=f"s_{h}_{i}")
                    for sc in range(2):
                        nc.tensor.matmul(s_[:, ds(sc * 512, 512)],
                                         k_sb[rh : rh + E, g, ts(i, P)],
                                         q_sb[rh : rh + E, g, ds(sc * 512, 512)],
                                         start=True, stop=True,
                                         tile_position=(rh, 0))
                    e_ = expp.tile([P, SQ], DT, tag="e", name=f"e_{h}_{i}")
                    nc.scalar.activation(e_[:], s_[:], Exp, bias=zb_sb[:])
                    if e_prev is not None:
                        for sc in range(2):
                            nc.tensor.matmul(cps[sc][:], v_sb[:, i - 1, vcols],
                                             e_prev[:, ds(sc * 512, 512)],
                                             start=i - 1 == 0, stop=False,
                                             tile_position=(0, 0))
                    for _ in range(2 if hpos == 0 else 1):
                        if queue:
                            queue.pop(0)()
                    e_prev = e_
                for sc in range(2):
                    nc.tensor.matmul(cps[sc][:], v_sb[:, NT - 1, vcols],
                                     e_prev[:, ds(sc * 512, 512)],
                                     start=False, stop=True, tile_position=(0, 0))
                for it in queue:
                    it()

                # drain ctx psum, compute 1/sum, broadcast, normalize
                ct = [ctmpp.tile([E + 1, 512], DT, tag=f"ct{sc}", name=f"ct{sc}_{h}")
                      for sc in range(2)]
                for sc in range(2):
                    nc.vector.tensor_copy(ct[sc][:], cps[sc][:])
                gr = bcp.tile([2, 1024], F32, tag="gr", name=f"gr_{h}")
                gath, rec = gr[:, 0:512], gr[:, 512:1024]
                for sc in range(2):
                    nc.gpsimd.dma_start(gath[sc : sc + 1, :], ct[sc][E : E + 1, :])
                nc.vector.reciprocal(rec[:], gath[:])
                rd = drp.tile([2, 512], F32, tag="rd", name=f"rd_{h}")
                nc.sync.dma_start(rd[:], rec[:])
                bc = bcp.tile([E, 1024], F32, tag="bc", name=f"bc_{h}")
                for sc in range(2):
                    nc.sync.dma_start(bc[:, ds(sc * 512, 512)],
                                      rd[sc : sc + 1, :].to_broadcast((E, 512)))
                if rh == 0:
                    for sc in range(2):
                        nc.vector.tensor_tensor(ctx_sb[0:E, g, ds(sc * 512, 512)],
                                                ct[sc][0:E, :],
                                                bc[:, ds(sc * 512, 512)], op=MUL)
                else:
                    bt = bcp.tile([E, 1024], DT, tag="btmp", name=f"bt_{h}")
                    for sc in range(2):
                        nc.vector.tensor_tensor(bt[:, ds(sc * 512, 512)],
                                                ct[sc][0:E, :],
                                                bc[:, ds(sc * 512, 512)], op=MUL)
                    nc.sync.dma_start(ctx_sb[E:P, g, :], bt[:])

            # ---- output projection: out[s, :] = ctx^T(s) @ wo + bias ----
            for j in range(NG):
                osb = osbp.tile([P, D], DT, tag="osb", name=f"osb_{j}")
                pso = [psP.tile([P, 512], F32, tag=f"pp{dc}", name=f"op_{j}_{dc}")
                       for dc in range(2)]
                for g in range(NG):
                    for dc in range(2):
                        nc.tensor.matmul(pso[dc][:],
                                         ctx_sb[:, g, ts(j, P)],
                                         wo_ref[0][:, g, ds(dc * 512, 512)],
                                         start=g == 0, stop=g == NG - 1)
                for dc in range(2):
                    nc.vector.tensor_tensor(osb[:, ds(dc * 512, 512)], pso[dc][:],
                                            bias_sb[:, ds(dc * 512, 512)], op=ADD)
                for dh in range(2):
                    nc.gpsimd.dma_start(out_d[ts(j, P), ds(dh * 512, 512)],
                                        osb[:, ds(dh * 512, 512)])

    nc.compile()
    return nc


def _get_nc():
    if "nc" not in _CACHE:
        _CACHE["nc"] = _build()
    return _CACHE["nc"]


def _prep_inputs(inputs):
    f32 = np.float32
    q = np.asarray(inputs["query"], f32)
    k = np.asarray(inputs["key"], f32)
    v = np.asarray(inputs["value"], f32)
    Wq = np.asarray(inputs["Wq"], f32)
    Wk = np.asarray(inputs["Wk"], f32)
    Wv = np.asarray(inputs["Wv"], f32)
    bq = np.asarray(inputs["bq"], f32).reshape(HE)
    bk = np.asarray(inputs["bk"], f32).reshape(HE)
    bv = np.asarray(inputs["bv"], f32)
    Wo = np.asarray(inputs["Wo"], f32)
    bo = np.asarray(inputs["bo"], f32)
    hs = np.asarray(inputs["head_selector"], f32)

    e = np.exp(hs - hs.max())
    hw = (e / e.sum()).astype(f32)

    scale = f32(1.0 / np.sqrt(E))
    wq2 = (Wq.transpose(1, 0, 2).reshape(D, HE) * scale)
    wk2 = Wk.transpose(1, 0, 2).reshape(D, HE)
    wv2 = (Wv * hw[:, None, None]).transpose(1, 0, 2).reshape(D, HE)
    bq2 = bq * scale
    bv2 = (bv * hw[:, None]).reshape(HE)
    bias_row = bv2 @ Wo + bo                       # [D]

    def chunked(w):  # [D, HE] -> [NG, D, 128] contiguous he-chunks
        return np.ascontiguousarray(
            w.reshape(D, NG, P).transpose(1, 0, 2)).astype(BF16)

    biases = np.zeros((P, D + 32), f32)
    biases[:, 0:D] = bias_row
    biases[:, D:D + NG] = bq2.reshape(NG, P).T
    biases[:, D + NG:D + 2 * NG] = bk.reshape(NG, P).T
    shared = {
        "wq": chunked(wq2),
        "wk": chunked(wk2),
        "wv": np.ascontiguousarray(wv2).astype(BF16),
        "wo": np.ascontiguousarray(Wo).astype(BF16),
        "biases": biases,
    }

    in_maps = []
    kT_b, vT_b = {}, {}
    for b in range(B):
        kT_b[b] = np.ascontiguousarray(k[b].T).astype(BF16)
        vT_b[b] = np.ascontiguousarray(v[b].T).astype(BF16)
    for c in range(8):
        b, half = c // 2, c % 2
        qT = np.ascontiguousarray(q[b, half * SQ:(half + 1) * SQ].T).astype(BF16)
        m = dict(shared)
        m["qT"] = qT
        m["kT"] = kT_b[b]
        m["vT"] = vT_b[b]
        in_maps.append(m)
    return in_maps


def _run(inputs, trace=False, trace_cores=None):
    from concourse.bass_utils import run_bass_kernel_spmd
    nc = _get_nc()
    in_maps = _prep_inputs(inputs)
    res = run_bass_kernel_spmd(nc, in_maps, core_ids=list(range(8)),
                               trace=trace, trace_cores=trace_cores)
    out = np.empty((B, S, D), np.float32)
    for c in range(8):
        b, half = c // 2, c % 2
        out[b, half * SQ:(half + 1) * SQ] = res.results[c]["out"]
    return out, res


def kernel(**inputs) -> np.ndarray:
    out, _ = _run(inputs, trace=False)
    return out
